# revision 20
# baseline (speedup 1.0000x reference)
"""Trainium2 Bass kernel for GQA attention (B=2, S=2048, HID=1024, 16 q / 4 kv
heads, HD=64, RoPE, causal softmax).

Sharding: 8 cores = 2 batches x 4 kv-head groups (tensor-parallel over kv
heads). Each core computes, for its (batch b, kv group g):
    Q_g = x_b @ wq_g.T (4 q heads), K/V_g = x_b @ wkv_g.T (1 kv head),
    RoPE, causal softmax attention, and a PARTIAL output projection
    out_partial = attn_g @ wo_g.T  (wo columns for group g).
Host sums the 4 partials per batch (partials stream back as bf16).

All matmul operands are bf16 (PSUM accumulates fp32). The kernel is one
fused loop over 512-query chunks: projections for chunk n+1 are interleaved
into the attention t-loop of chunk n so the tensor engine never idles (keeps
the HAM clock gate at K=8/8), then the chunk's output projection streams out.
Softmax denominators ride as a ones column in the PV matmul ([V|1|V] layout
gives both head-halves aligned partitions with no shift matmul); reciprocal
on the vector engine; gpsimd partition-broadcast replicates it across the
head dim. Exp runs on the scalar engine, fp32-in / bf16-out.
"""
import sys

sys.path.insert(0, "/opt/trn_rl_repo")

import numpy as np
import ml_dtypes
from contextlib import ExitStack

import concourse.bass as bass
import concourse.tile as tile
from concourse import bacc
from concourse import mybir
from concourse.bass_utils import run_bass_kernel_spmd

# problem constants (hardcoded per contract)
B, S, HID = 2, 2048, 1024
NH, NKV, HD = 16, 4, 64
P = 128
NK = HID // P          # 8 k-tiles over hidden
NQC = S // 512         # 4 q-chunks of 512
NST = S // P           # 16 seq tiles of 128
QH = NH // NKV         # 4 q heads per core
FEAT = QH * HD         # 256 features per core
MASK_NEG = -1e9
VW = 2 * HD + 1        # 129: [V | ones | V] augmented value layout

F32 = mybir.dt.float32
BF16 = mybir.dt.bfloat16
BF = ml_dtypes.bfloat16
EXP = mybir.ActivationFunctionType.Exp


def _pin_act_tables():
    # Keep table ids aligned with act_info.json but make every set except
    # natural_log_exp_and_others ineligible, so Exp/Ln/Copy all resolve to
    # one table and bacc hoists a single ACT_TABLE_LOAD.
    import concourse.hw_specs as hw_specs
    import concourse.bacc as bacc_mod
    real = hw_specs.get_activation_tables

    def pinned(arch):
        tabs = dict(real(arch))
        return {name: (funcs if name == "natural_log_exp_and_others" else set())
                for name, funcs in tabs.items()}

    bacc_mod.get_activation_tables = pinned


def build_program():
    _pin_act_tables()
    nc = bacc.Bacc("TRN2", target_bir_lowering=False, debug=False)

    d_xT = nc.dram_tensor("xT", [HID, S], BF16, kind="ExternalInput").ap()
    d_wqT = nc.dram_tensor("wqT", [HID, FEAT], BF16, kind="ExternalInput").ap()
    d_wkvT = nc.dram_tensor("wkvT", [HID, 2 * HD], BF16, kind="ExternalInput").ap()
    d_woT = nc.dram_tensor("woT", [FEAT, HID], BF16, kind="ExternalInput").ap()
    d_cosT = nc.dram_tensor("cosT", [P, S], BF16, kind="ExternalInput").ap()
    d_sinT = nc.dram_tensor("sinT", [P, S], BF16, kind="ExternalInput").ap()
    d_r128 = nc.dram_tensor("r128", [P, P], BF16, kind="ExternalInput").ap()
    d_rdup = nc.dram_tensor("rdup", [HD, P], BF16, kind="ExternalInput").ap()
    d_idup = nc.dram_tensor("idup", [HD, P], BF16, kind="ExternalInput").ap()
    d_ident = nc.dram_tensor("ident", [P, HD], BF16, kind="ExternalInput").ap()
    d_ishift = nc.dram_tensor("ishift", [HD, P], BF16, kind="ExternalInput").ap()
    d_tri = nc.dram_tensor("tri", [P, P], BF16, kind="ExternalInput").ap()
    d_i128b = nc.dram_tensor("i128b", [P, P], BF16, kind="ExternalInput").ap()
    d_ones1 = nc.dram_tensor("ones1c", [1, HD], BF16, kind="ExternalInput").ap()
    d_onesv = nc.dram_tensor("onesv", [P, NST], BF16, kind="ExternalInput").ap()
    d_out = nc.dram_tensor("outp", [S, HID], BF16, kind="ExternalOutput").ap()

    with tile.TileContext(nc) as tc, ExitStack() as ctx, \
            nc.allow_low_precision(reason="bf16 compute within tolerance"):
        consts = ctx.enter_context(tc.tile_pool(name="consts", bufs=1))
        main = ctx.enter_context(tc.tile_pool(name="main", bufs=1))
        sbp = ctx.enter_context(tc.tile_pool(name="sbp", bufs=2))
        qpl = ctx.enter_context(tc.tile_pool(name="qpl", bufs=2))
        atp = ctx.enter_context(tc.tile_pool(name="atp", bufs=2))
        ptp = ctx.enter_context(tc.tile_pool(name="ptp", bufs=4))
        xpool = ctx.enter_context(tc.tile_pool(name="xt", bufs=9))
        otp = ctx.enter_context(tc.tile_pool(name="otp", bufs=3))
        # PSUM: sc 2x[128,2,512]f32 (4 banks) + pj 1x[128,1024]f32 (2 banks)
        #       + pv 2x[128,512]f32 (2 banks) = 8 banks exactly
        scps = ctx.enter_context(tc.tile_pool(name="scps", bufs=2, space="PSUM"))
        pjps = ctx.enter_context(tc.tile_pool(name="pjps", bufs=1, space="PSUM"))
        pvps = ctx.enter_context(tc.tile_pool(name="pvps", bufs=2, space="PSUM"))

        # ---- constants to SBUF
        wq_sb = consts.tile([P, NK, FEAT], BF16)
        nc.scalar.dma_start(wq_sb[:], d_wqT.rearrange("(ko p) m -> p ko m", p=P))
        wkv_sb = consts.tile([P, NK, 2 * HD], BF16)
        nc.scalar.dma_start(wkv_sb[:], d_wkvT.rearrange("(ko p) m -> p ko m", p=P))
        wo_sb = consts.tile([P, 2, HID], BF16)
        nc.gpsimd.dma_start(wo_sb[:], d_woT.rearrange("(ko p) m -> p ko m", p=P))
        cos_sb = consts.tile([P, S], BF16)
        nc.scalar.dma_start(cos_sb[:], d_cosT)
        sin_sb = consts.tile([P, S], BF16)
        nc.gpsimd.dma_start(sin_sb[:], d_sinT)
        r128_sb = consts.tile([P, P], BF16)
        nc.sync.dma_start(r128_sb[:], d_r128)
        rdup_sb = consts.tile([HD, P], BF16)
        nc.sync.dma_start(rdup_sb[:], d_rdup)
        idup_sb = consts.tile([HD, P], BF16)
        nc.sync.dma_start(idup_sb[:], d_idup)
        ident_sb = consts.tile([P, HD], BF16)
        nc.sync.dma_start(ident_sb[:], d_ident)
        ishift_sb = consts.tile([HD, P], BF16)
        nc.sync.dma_start(ishift_sb[:], d_ishift)
        tri_sb = consts.tile([P, P], BF16)
        nc.sync.dma_start(tri_sb[:], d_tri)
        i128b_sb = consts.tile([P, P], BF16)
        nc.sync.dma_start(i128b_sb[:], d_i128b)

        ones1_sb = consts.tile([1, HD], BF16)
        nc.sync.dma_start(ones1_sb[:], d_ones1)

        # ---- persistent activations
        kpt = main.tile([P, S], BF16)             # roped K^T, duplicated halves
        vaug = main.tile([P, NST, HD + 1], BF16)  # [V | ones] per seq tile
        nc.sync.dma_start(vaug[:, :, HD:HD + 1], d_onesv)

        # ---- PE warm-up burst to flip HAM to K=8/8 at kernel start
        wp = pvps.tile([P, 512], F32, tag="pv", name="warm")
        for _ in range(48):
            nc.tensor.matmul(wp[:, 0:P], i128b_sb[:], i128b_sb[:],
                             start=True, stop=True, skip_group_check=True)

        refs = {}

        def prepare(n):
            """Generator: projections + RoPE + V transpose for chunk n.

            Yields after small PE bundles so the caller can interleave these
            into the attention t-loop of chunk n-1.
            """
            c0 = n * 512
            xts = []
            for k in range(NK):
                xt = xpool.tile([P, 512], BF16, tag="xt", name=f"x{n}_{k}")
                eng = (nc.sync, nc.gpsimd)[k % 2]
                eng.dma_start(xt[:], d_xT[k * P:(k + 1) * P, c0:c0 + 512])
                xts.append(xt)
            # Q projection
            ps_qq = pjps.tile([P, 1024], F32, tag="pj", name=f"qq{n}")
            for k in range(NK):
                nc.tensor.matmul(ps_qq[:, 0:512], wq_sb[:, k, 0:P], xts[k][:],
                                 start=(k == 0), stop=(k == NK - 1),
                                 skip_group_check=True)
                nc.tensor.matmul(ps_qq[:, 512:1024], wq_sb[:, k, P:FEAT],
                                 xts[k][:], start=(k == 0), stop=(k == NK - 1),
                                 skip_group_check=True)
                yield
            qraw = sbp.tile([P, 1024], BF16, tag="qraw", name=f"qr{n}")
            nc.vector.tensor_copy(qraw[:], ps_qq[:])
            yield
            ps_rot = pjps.tile([P, 1024], F32, tag="pj", name=f"rot{n}")
            nc.tensor.matmul(ps_rot[:, 0:512], r128_sb[:], qraw[:, 0:512],
                             start=True, stop=True, skip_group_check=True)
            yield
            nc.tensor.matmul(ps_rot[:, 512:1024], r128_sb[:], qraw[:, 512:1024],
                             start=True, stop=True, skip_group_check=True)
            yield
            qn = qpl.tile([P, 2, 512], BF16, tag="qn", name=f"q{n}")
            refs[n] = qn
            cs = cos_sb[:, c0:c0 + 512]
            sn = sin_sb[:, c0:c0 + 512]
            for m in (0, 1):
                t1 = sbp.tile([P, 512], BF16, tag="t1", name=f"t1q{n}{m}")
                nc.vector.tensor_mul(t1[:], qraw[:, m * 512:(m + 1) * 512], cs)
                t2 = sbp.tile([P, 512], BF16, tag="t2", name=f"t2q{n}{m}")
                nc.vector.tensor_mul(t2[:], ps_rot[:, m * 512:(m + 1) * 512], sn)
                nc.gpsimd.tensor_add(qn[:, m, :], t1[:], t2[:])
                yield
            # KV projection
            ps_kv = pjps.tile([P, 1024], F32, tag="pj", name=f"kv{n}")
            for k in range(NK):
                nc.tensor.matmul(ps_kv[:, 0:512], wkv_sb[:, k, :], xts[k][:],
                                 start=(k == 0), stop=(k == NK - 1),
                                 skip_group_check=True)
                if k % 2 == 1:
                    yield
            kvraw = sbp.tile([P, 512], BF16, tag="kvr", name=f"kvr{n}")
            nc.vector.tensor_copy(kvraw[:], ps_kv[:, 0:512])
            yield
            ps_kr = pjps.tile([P, 1024], F32, tag="pj", name=f"kr{n}")
            nc.tensor.matmul(ps_kr[:, 0:512], idup_sb[:], kvraw[0:HD, :],
                             start=True, stop=True, skip_group_check=True)
            yield
            nc.tensor.matmul(ps_kr[:, 512:1024], rdup_sb[:], kvraw[0:HD, :],
                             start=True, stop=True, skip_group_check=True)
            yield
            t1 = sbp.tile([P, 512], BF16, tag="t1", name=f"t1k{n}")
            nc.vector.tensor_mul(t1[:], ps_kr[:, 0:512], cs)
            t2 = sbp.tile([P, 512], BF16, tag="t2", name=f"t2k{n}")
            nc.vector.tensor_mul(t2[:], ps_kr[:, 512:1024], sn)
            nc.gpsimd.tensor_add(kpt[:, c0:c0 + 512], t1[:], t2[:])
            yield
            # V transposes
            ps_vt = pjps.tile([P, 4, HD], BF16, tag="pj", name=f"vt{n}")
            for tt in range(4):
                nc.tensor.matmul(ps_vt[:, tt, :],
                                 kvraw[HD:P, tt * P:(tt + 1) * P],
                                 ident_sb[HD:P, :], is_transpose=True,
                                 skip_group_check=True)
                yield
            nc.vector.tensor_copy(vaug[:, 4 * n:4 * n + 4, 0:HD], ps_vt[:])
            yield

        def drain(gen, k=1):
            if gen is None:
                return None
            for _ in range(k):
                if next(gen, "END") == "END":
                    return None
            return gen

        def attention(n, m, qn, at_n, gen):
            """Attention for chunk n, head pair m; interleaves gen steps."""
            T = 4 * n + 4
            pv0 = pvps.tile([P, 512], F32, tag="pv", name=f"pv0_{n}{m}")
            pv1 = pvps.tile([P, 512], F32, tag="pv", name=f"pv1_{n}{m}")
            sc_t = {}

            def emit_scores(t):
                r = t - 4 * n
                lo = P * r if r >= 0 else 0
                sc = scps.tile([P, 2, 512], F32, tag="sc", name=f"sc{n}{m}{t}")
                for h2 in (0, 1):
                    kl = kpt[h2 * HD:(h2 + 1) * HD, t * P:(t + 1) * P]
                    ql = qn[h2 * HD:(h2 + 1) * HD, m, lo:512]
                    if r >= 0:
                        nc.tensor.matmul(sc[:, h2, lo:512], kl, ql,
                                         start=True, stop=False,
                                         skip_group_check=True)
                        nc.tensor.matmul(sc[:, h2, lo:lo + P], i128b_sb[:],
                                         tri_sb[:], start=False, stop=True,
                                         skip_group_check=True)
                    else:
                        nc.tensor.matmul(sc[:, h2, :], kl, ql,
                                         start=True, stop=True,
                                         skip_group_check=True)
                sc_t[t] = sc

            emit_scores(0)
            for t in range(T):
                if t + 1 < T:
                    emit_scores(t + 1)
                r = t - 4 * n
                lo = P * r if r >= 0 else 0
                sc = sc_t.pop(t)
                pt = ptp.tile([P, 2, 512], BF16, tag="pt", name=f"p{n}{m}{t}")
                if lo == 0:
                    nc.scalar.activation(pt[:], sc[:], EXP, scale=0.125)
                else:
                    nc.scalar.activation(pt[:, :, lo:512], sc[:, :, lo:512],
                                         EXP, scale=0.125)
                gen = drain(gen)
                for h2, pv in ((0, pv0), (1, pv1)):
                    nc.tensor.matmul(pv[0:HD + 1, lo:512], vaug[:, t, :],
                                     pt[:, h2, lo:512],
                                     start=(t == 0), stop=(t == T - 1),
                                     skip_group_check=True)
            # epilogue: normalize via the ones-column sums (both pv tiles
            # carry data rows 0..63 and the denominator at row 64)
            dsa = sbp.tile([1, 512], F32, tag="dsa", name=f"da{n}{m}")
            nc.scalar.copy(dsa[:], pv0[HD:HD + 1, :])
            dsb = sbp.tile([1, 512], F32, tag="dsb", name=f"db{n}{m}")
            nc.scalar.copy(dsb[:], pv1[HD:HD + 1, :])
            reca = sbp.tile([1, 512], BF16, tag="reca", name=f"ra{n}{m}")
            nc.vector.reciprocal(reca[:], dsa[:])
            recb = sbp.tile([1, 512], BF16, tag="recb", name=f"rb{n}{m}")
            nc.vector.reciprocal(recb[:], dsb[:])
            rbps = scps.tile([P, 2, 512], F32, tag="sc", name=f"rb{n}{m}")
            nc.tensor.matmul(rbps[0:HD, 0, :], ones1_sb[:], reca[:],
                             start=True, stop=True, skip_group_check=True)
            nc.tensor.matmul(rbps[0:HD, 1, :], ones1_sb[:], recb[:],
                             start=True, stop=True, skip_group_check=True)
            rec_s = sbp.tile([HD, 2, 512], BF16, tag="recs", name=f"rs{n}{m}")
            nc.vector.tensor_copy(rec_s[:], rbps[0:HD, :, :])
            nc.vector.tensor_mul(at_n[0:HD, m, :], pv0[0:HD, :],
                                 rec_s[:, 0, :])
            # odd half: scale at partitions 0..63, then shift to 64..127 via
            # the ishift matmul (PSUM col offsets must be 0/64-aligned)
            tmp = sbp.tile([HD, 512], BF16, tag="oddt", name=f"od{n}{m}")
            nc.vector.tensor_mul(tmp[:], pv1[0:HD, :], rec_s[:, 1, :])
            rp = scps.tile([P, 2, 512], F32, tag="sc", name=f"rp{n}{m}")
            nc.tensor.matmul(rp[:, 0, :], ishift_sb[:], tmp[:],
                             start=True, stop=True, skip_group_check=True)
            nc.vector.tensor_copy(at_n[HD:P, m, :], rp[HD:P, 0, :])
            return gen

        def out_proj(n, at_n):
            for st in range(4):
                po = pjps.tile([P, 1024], F32, tag="pj", name=f"po{n}{st}")
                sl = slice(st * P, (st + 1) * P)
                for nn2 in (0, 1):
                    for m in (0, 1):
                        nc.tensor.matmul(po[:, nn2 * 512:(nn2 + 1) * 512],
                                         at_n[:, m, sl],
                                         wo_sb[:, m, nn2 * 512:(nn2 + 1) * 512],
                                         start=(m == 0), stop=(m == 1),
                                         skip_group_check=True)
                ot = otp.tile([P, 1024], BF16, tag="ot", name=f"o{n}{st}")
                if st % 2 == 0:
                    nc.vector.tensor_copy(ot[:], po[:])
                else:
                    nc.scalar.copy(ot[:], po[:])
                nc.sync.dma_start(d_out[(4 * n + st) * P:(4 * n + st + 1) * P, :],
                                  ot[:])

        # ---- fused main loop
        gen = prepare(0)
        while drain(gen) is not None:
            pass
        gen = None
        for n in range(NQC):
            if n + 1 < NQC:
                gen = prepare(n + 1)
            at_n = atp.tile([P, 2, 512], BF16, tag="at", name=f"at{n}")
            for m in (0, 1):
                gen = attention(n, m, refs[n], at_n, gen)
            while drain(gen) is not None:
                pass
            gen = None
            out_proj(n, at_n)

    nc.compile()
    return nc


def make_consts():
    """Host-precomputed constant operands shared by all cores."""
    r128 = np.zeros((P, P), np.float32)
    for mm in range(P):
        hh, dd = mm // HD, mm % HD
        if dd < HD // 2:
            r128[hh * HD + dd + HD // 2, mm] = -1.0
        else:
            r128[hh * HD + dd - HD // 2, mm] = 1.0
    rdup = np.zeros((HD, P), np.float32)
    idup = np.zeros((HD, P), np.float32)
    for mm in range(P):
        dd = mm % HD
        idup[dd, mm] = 1.0
        if dd < HD // 2:
            rdup[dd + HD // 2, mm] = -1.0
        else:
            rdup[dd - HD // 2, mm] = 1.0
    ident = np.zeros((P, HD), np.float32)
    ident[HD:P, :] = np.eye(HD)
    ishift = np.zeros((HD, P), np.float32)
    for kk in range(HD):
        ishift[kk, kk + HD] = 1.0
    tri = np.where(np.arange(P)[:, None] <= np.arange(P)[None, :], 0.0,
                   MASK_NEG).astype(BF)
    return dict(r128=r128.astype(BF), rdup=rdup.astype(BF),
                idup=idup.astype(BF), ident=ident.astype(BF),
                ishift=ishift.astype(BF),
                tri=tri, i128b=np.eye(P).astype(BF),
                ones1c=np.ones((1, HD), BF),
                onesv=np.ones((P, NST), BF))


def prep_in_maps(x, cos, sin, wq, wk, wv, wo):
    """Per-core input maps (shared by kernel() and the test harness)."""
    consts = make_consts()
    cosT = np.ascontiguousarray(np.vstack([cos.T, cos.T])).astype(BF)  # [128,S]
    sinT = np.ascontiguousarray(np.vstack([sin.T, sin.T])).astype(BF)

    in_maps = []
    for core in range(8):
        b, g = core // NKV, core % NKV
        xT = np.ascontiguousarray(x[b].T).astype(BF)                 # [HID, S]
        wqT = np.ascontiguousarray(wq[g * FEAT:(g + 1) * FEAT, :].T).astype(BF)
        wkvT = np.ascontiguousarray(
            np.concatenate([wk[g * HD:(g + 1) * HD, :],
                            wv[g * HD:(g + 1) * HD, :]], axis=0).T).astype(BF)
        woT = np.ascontiguousarray(wo[:, g * FEAT:(g + 1) * FEAT].T).astype(BF)
        in_maps.append(dict(xT=xT, wqT=wqT, wkvT=wkvT, woT=woT,
                            cosT=cosT, sinT=sinT, **consts))
    return in_maps


_PROG = None


def kernel(x, cos, sin, wq, wk, wv, wo):
    global _PROG
    x = np.asarray(x, np.float32)
    cos = np.asarray(cos, np.float32)
    sin = np.asarray(sin, np.float32)
    wq = np.asarray(wq, np.float32)
    wk = np.asarray(wk, np.float32)
    wv = np.asarray(wv, np.float32)
    wo = np.asarray(wo, np.float32)

    in_maps = prep_in_maps(x, cos, sin, wq, wk, wv, wo)

    if _PROG is None:
        _PROG = build_program()
    res = run_bass_kernel_spmd(_PROG, in_maps, core_ids=list(range(8)))

    out = np.zeros((B, S, HID), np.float32)
    for core in range(8):
        out[core // NKV] += np.asarray(res.results[core]["outp"], np.float32)
    return out


if __name__ == "__main__":
    rng = np.random.default_rng(0)
    ins = dict(
        x=rng.standard_normal((B, S, HID), np.float32),
        cos=rng.random((S, HD), np.float32),
        sin=rng.random((S, HD), np.float32),
        wq=rng.standard_normal((HID, HID), np.float32) * HID ** -0.5,
        wk=rng.standard_normal((NKV * HD, HID), np.float32) * HID ** -0.5,
        wv=rng.standard_normal((NKV * HD, HID), np.float32) * HID ** -0.5,
        wo=rng.standard_normal((HID, HID), np.float32) * HID ** -0.5,
    )
    out = kernel(**ins)
    print("kernel ran, out shape", out.shape, "mean", float(np.abs(out).mean()))


# revision 32
# speedup vs baseline: 1.2747x; 1.2747x over previous
"""Trainium2 Bass kernel for GQA attention (B=2, S=2048, HID=1024, 16 q / 4 kv
heads, HD=64, RoPE, causal softmax).

Sharding: 8 cores = 2 batches x 4 kv-head groups (tensor-parallel over kv
heads). Each core computes, for its (batch b, kv group g):
    Q_g = x_b @ wq_g.T (4 q heads), K/V_g = x_b @ wkv_g.T (1 kv head),
    RoPE, causal softmax attention, and a PARTIAL output projection
    out_partial = attn_g @ wo_g.T  (wo columns for group g).
Host sums the 4 partials per batch (partials stream back as bf16).

All matmul operands are bf16 (PSUM accumulates fp32). The kernel is one
fused loop over 512-query chunks: projections for chunk n+1 are interleaved
into the attention t-loop of chunk n so the tensor engine never idles (keeps
the HAM clock gate at K=8/8), then the chunk's output projection streams out.
Softmax denominators ride as a ones column in the PV matmul ([V|1|V] layout
gives both head-halves aligned partitions with no shift matmul); reciprocal
on the vector engine; gpsimd partition-broadcast replicates it across the
head dim. Exp runs on the scalar engine, fp32-in / bf16-out.
"""
import sys

sys.path.insert(0, "/opt/trn_rl_repo")

import numpy as np
import ml_dtypes
from contextlib import ExitStack

import concourse.bass as bass
import concourse.tile as tile
from concourse import bacc
from concourse import mybir
from concourse.bass_utils import run_bass_kernel_spmd

# problem constants (hardcoded per contract)
B, S, HID = 2, 2048, 1024
NH, NKV, HD = 16, 4, 64
P = 128
NK = HID // P          # 8 k-tiles over hidden
NQC = S // 512         # 4 q-chunks of 512
NST = S // P           # 16 seq tiles of 128
QH = NH // NKV         # 4 q heads per core
FEAT = QH * HD         # 256 features per core
MASK_NEG = -1e9
VW = 2 * HD + 1        # 129: [V | ones | V] augmented value layout

F32 = mybir.dt.float32
F32R = mybir.dt.float32r
BF16 = mybir.dt.bfloat16
BF = ml_dtypes.bfloat16
EXP = mybir.ActivationFunctionType.Exp


def _pin_act_tables():
    # Keep table ids aligned with act_info.json but make every set except
    # natural_log_exp_and_others ineligible, so Exp/Ln/Copy all resolve to
    # one table and bacc hoists a single ACT_TABLE_LOAD.
    import concourse.hw_specs as hw_specs
    import concourse.bacc as bacc_mod
    real = hw_specs.get_activation_tables

    def pinned(arch):
        tabs = dict(real(arch))
        return {name: (funcs if name == "natural_log_exp_and_others" else set())
                for name, funcs in tabs.items()}

    bacc_mod.get_activation_tables = pinned


def build_program():
    _pin_act_tables()
    nc = bacc.Bacc("TRN2", target_bir_lowering=False, debug=False)

    d_xT = nc.dram_tensor("xT", [HID, S], BF16, kind="ExternalInput").ap()
    d_wqT = nc.dram_tensor("wqT", [HID, FEAT], BF16, kind="ExternalInput").ap()
    d_wkvT = nc.dram_tensor("wkvT", [HID, 2 * HD], BF16, kind="ExternalInput").ap()
    d_woT = nc.dram_tensor("woT", [FEAT, HID], BF16, kind="ExternalInput").ap()
    d_cosT = nc.dram_tensor("cosT", [P, S], BF16, kind="ExternalInput").ap()
    d_sinT = nc.dram_tensor("sinT", [P, S], BF16, kind="ExternalInput").ap()
    d_r128 = nc.dram_tensor("r128", [P, P], BF16, kind="ExternalInput").ap()
    d_rdup = nc.dram_tensor("rdup", [HD, P], BF16, kind="ExternalInput").ap()
    d_idup = nc.dram_tensor("idup", [HD, P], BF16, kind="ExternalInput").ap()
    d_ident = nc.dram_tensor("ident", [P, HD], BF16, kind="ExternalInput").ap()
    d_ishift = nc.dram_tensor("ishift", [HD, P], BF16, kind="ExternalInput").ap()
    d_tri = nc.dram_tensor("tri", [P, P], BF16, kind="ExternalInput").ap()
    d_i128b = nc.dram_tensor("i128b", [P, P], BF16, kind="ExternalInput").ap()
    d_ones1 = nc.dram_tensor("ones1c", [P, HD], BF16, kind="ExternalInput").ap()
    d_onesv = nc.dram_tensor("onesv", [P, NST], BF16, kind="ExternalInput").ap()
    d_out = nc.dram_tensor("outp", [S, HID], BF16, kind="ExternalOutput").ap()

    with tile.TileContext(nc) as tc, ExitStack() as ctx, \
            nc.allow_low_precision(reason="bf16 compute within tolerance"):
        consts = ctx.enter_context(tc.tile_pool(name="consts", bufs=1))
        main = ctx.enter_context(tc.tile_pool(name="main", bufs=1))
        sbp = ctx.enter_context(tc.tile_pool(name="sbp", bufs=2))
        qpl = ctx.enter_context(tc.tile_pool(name="qpl", bufs=2))
        atp = ctx.enter_context(tc.tile_pool(name="atp", bufs=2))
        ptp = ctx.enter_context(tc.tile_pool(name="ptp", bufs=4))
        xpool = ctx.enter_context(tc.tile_pool(name="xt", bufs=9))
        otp = ctx.enter_context(tc.tile_pool(name="otp", bufs=3))
        # PSUM: sc 2x[128,2,512]f32 (4 banks) + pj 1x[128,1024]f32 (2 banks)
        #       + pv 2x[128,512]f32 (2 banks) = 8 banks exactly
        scps = ctx.enter_context(tc.tile_pool(name="scps", bufs=2, space="PSUM"))
        pjps = ctx.enter_context(tc.tile_pool(name="pjps", bufs=1, space="PSUM"))
        pvps = ctx.enter_context(tc.tile_pool(name="pvps", bufs=2, space="PSUM"))

        # ---- constants to SBUF
        wq_sb = consts.tile([P, NK, FEAT], BF16)
        nc.scalar.dma_start(wq_sb[:], d_wqT.rearrange("(ko p) m -> p ko m", p=P))
        wkv_sb = consts.tile([P, NK, 2 * HD], BF16)
        nc.scalar.dma_start(wkv_sb[:], d_wkvT.rearrange("(ko p) m -> p ko m", p=P))
        wo_sb = consts.tile([P, 2, HID], BF16)
        nc.gpsimd.dma_start(wo_sb[:], d_woT.rearrange("(ko p) m -> p ko m", p=P))
        cos_sb = consts.tile([P, S], BF16)
        nc.scalar.dma_start(cos_sb[:], d_cosT)
        sin_sb = consts.tile([P, S], BF16)
        nc.gpsimd.dma_start(sin_sb[:], d_sinT)
        r128_sb = consts.tile([P, P], BF16)
        nc.sync.dma_start(r128_sb[:], d_r128)
        rdup_sb = consts.tile([HD, P], BF16)
        nc.sync.dma_start(rdup_sb[:], d_rdup)
        idup_sb = consts.tile([HD, P], BF16)
        nc.sync.dma_start(idup_sb[:], d_idup)
        ident_sb = consts.tile([P, HD], BF16)
        nc.sync.dma_start(ident_sb[:], d_ident)
        ishift_sb = consts.tile([HD, P], BF16)
        nc.sync.dma_start(ishift_sb[:], d_ishift)
        tri_sb = consts.tile([P, P], BF16)
        nc.sync.dma_start(tri_sb[:], d_tri)
        i128b_sb = consts.tile([P, P], BF16)
        nc.sync.dma_start(i128b_sb[:], d_i128b)

        ones1_sb = consts.tile([P, HD], BF16)
        nc.sync.dma_start(ones1_sb[:], d_ones1)

        # ---- persistent activations
        kpt = main.tile([P, S], BF16)             # roped K^T, duplicated halves
        vaug = main.tile([P, NST, HD + 1], BF16)  # [V | ones] per seq tile
        nc.sync.dma_start(vaug[:, :, HD:HD + 1], d_onesv)

        # ---- PE warm-up burst to flip HAM to K=8/8 at kernel start
        wp = pvps.tile([P, 512], F32, tag="pv", name="warm")
        for _ in range(48):
            nc.tensor.matmul(wp[:, 0:P], i128b_sb[:], i128b_sb[:],
                             start=True, stop=True, skip_group_check=True)

        refs = {}

        def prepare(n):
            """Generator: projections + RoPE + V transpose for chunk n.

            Yields after small PE bundles so the caller can interleave these
            into the attention t-loop of chunk n-1.
            """
            c0 = n * 512
            xts = []
            for k in range(NK):
                xt = xpool.tile([P, 512], BF16, tag="xt", name=f"x{n}_{k}")
                eng = (nc.sync, nc.gpsimd)[k % 2]
                eng.dma_start(xt[:], d_xT[k * P:(k + 1) * P, c0:c0 + 512])
                xts.append(xt)
            # Q projection
            ps_qq = pjps.tile([P, 1024], F32, tag="pj", name=f"qq{n}")
            for k in range(NK):
                nc.tensor.matmul(ps_qq[:, 0:512], wq_sb[:, k, 0:P], xts[k][:],
                                 start=(k == 0), stop=(k == NK - 1),
                                 skip_group_check=True)
                nc.tensor.matmul(ps_qq[:, 512:1024], wq_sb[:, k, P:FEAT],
                                 xts[k][:], start=(k == 0), stop=(k == NK - 1),
                                 skip_group_check=True)
                yield
            qraw = sbp.tile([P, 1024], BF16, tag="qraw", name=f"qr{n}")
            nc.vector.tensor_copy(qraw[:], ps_qq[:])
            yield
            ps_rot = pjps.tile([P, 1024], F32, tag="pj", name=f"rot{n}")
            nc.tensor.matmul(ps_rot[:, 0:512], r128_sb[:], qraw[:, 0:512],
                             start=True, stop=True, skip_group_check=True)
            yield
            nc.tensor.matmul(ps_rot[:, 512:1024], r128_sb[:], qraw[:, 512:1024],
                             start=True, stop=True, skip_group_check=True)
            yield
            qn = qpl.tile([P, 2, 512], BF16, tag="qn", name=f"q{n}")
            refs[n] = qn
            cs = cos_sb[:, c0:c0 + 512]
            sn = sin_sb[:, c0:c0 + 512]
            for m in (0, 1):
                t1 = sbp.tile([P, 512], BF16, tag="t1", name=f"t1q{n}{m}")
                nc.vector.tensor_mul(t1[:], qraw[:, m * 512:(m + 1) * 512], cs)
                t2 = sbp.tile([P, 512], BF16, tag="t2", name=f"t2q{n}{m}")
                nc.vector.tensor_mul(t2[:], ps_rot[:, m * 512:(m + 1) * 512], sn)
                nc.gpsimd.tensor_add(qn[:, m, :], t1[:], t2[:])
                yield
            # KV projection
            ps_kv = pjps.tile([P, 1024], F32, tag="pj", name=f"kv{n}")
            for k in range(NK):
                nc.tensor.matmul(ps_kv[:, 0:512], wkv_sb[:, k, :], xts[k][:],
                                 start=(k == 0), stop=(k == NK - 1),
                                 skip_group_check=True)
                if k % 2 == 1:
                    yield
            kvraw = sbp.tile([P, 512], BF16, tag="kvr", name=f"kvr{n}")
            nc.vector.tensor_copy(kvraw[:], ps_kv[:, 0:512])
            yield
            ps_kr = pjps.tile([P, 1024], F32, tag="pj", name=f"kr{n}")
            nc.tensor.matmul(ps_kr[:, 0:512], idup_sb[:], kvraw[0:HD, :],
                             start=True, stop=True, skip_group_check=True)
            yield
            nc.tensor.matmul(ps_kr[:, 512:1024], rdup_sb[:], kvraw[0:HD, :],
                             start=True, stop=True, skip_group_check=True)
            yield
            t1 = sbp.tile([P, 512], BF16, tag="t1", name=f"t1k{n}")
            nc.vector.tensor_mul(t1[:], ps_kr[:, 0:512], cs)
            t2 = sbp.tile([P, 512], BF16, tag="t2", name=f"t2k{n}")
            nc.vector.tensor_mul(t2[:], ps_kr[:, 512:1024], sn)
            nc.gpsimd.tensor_add(kpt[:, c0:c0 + 512], t1[:], t2[:])
            yield
            # V transposes
            ps_vt = pjps.tile([P, 4, HD], BF16, tag="pj", name=f"vt{n}")
            for tt in range(4):
                nc.tensor.matmul(ps_vt[:, tt, :],
                                 kvraw[HD:P, tt * P:(tt + 1) * P],
                                 ident_sb[HD:P, :], is_transpose=True,
                                 skip_group_check=True)
                yield
            nc.vector.tensor_copy(vaug[:, 4 * n:4 * n + 4, 0:HD], ps_vt[:])
            yield

        def drain(gen, k=1):
            if gen is None:
                return None
            for _ in range(k):
                if next(gen, "END") == "END":
                    return None
            return gen

        def attention(n, m, qn, at_n, gen, epi_prev):
            """Attention for chunk n, head pair m; interleaves gen steps.

            Returns (gen, epi_b): epi_b is a closure emitting the PE half of
            this head pair's softmax-normalize epilogue; the caller defers it
            into the NEXT attention's instruction stream so the PE queue has
            independent work in front of it while the DVE/scalar half runs.
            """
            T = 4 * n + 4
            pv0 = pvps.tile([P, 512], F32, tag="pv", name=f"pv0_{n}{m}")
            pv1 = pvps.tile([P, 512], F32, tag="pv", name=f"pv1_{n}{m}")
            sc_t = {}

            def emit_scores(t):
                r = t - 4 * n
                lo = P * r if r >= 0 else 0
                sc = scps.tile([P, 2, 512], F32, tag="sc", name=f"sc{n}{m}{t}")
                for h2 in (0, 1):
                    kl = kpt[h2 * HD:(h2 + 1) * HD, t * P:(t + 1) * P]
                    ql = qn[h2 * HD:(h2 + 1) * HD, m, lo:512]
                    if r >= 0:
                        nc.tensor.matmul(sc[:, h2, lo:512], kl, ql,
                                         start=True, stop=False,
                                         skip_group_check=True)
                        nc.tensor.matmul(sc[:, h2, lo:lo + P], i128b_sb[:],
                                         tri_sb[:], start=False, stop=True,
                                         skip_group_check=True)
                    else:
                        nc.tensor.matmul(sc[:, h2, :], kl, ql,
                                         start=True, stop=True,
                                         skip_group_check=True)
                sc_t[t] = sc

            emit_scores(0)
            if T > 1:
                emit_scores(1)
            if epi_prev is not None:
                epi_prev()
            for t in range(T):
                if t + 2 < T:
                    emit_scores(t + 2)
                r = t - 4 * n
                lo = P * r if r >= 0 else 0
                sc = sc_t.pop(t)
                pt = ptp.tile([P, 2, 512], BF16, tag="pt", name=f"p{n}{m}{t}")
                if lo == 0:
                    nc.scalar.activation(pt[:], sc[:], EXP, scale=0.125)
                else:
                    nc.scalar.activation(pt[:, :, lo:512], sc[:, :, lo:512],
                                         EXP, scale=0.125)
                gen = drain(gen)
                for h2, pv in ((0, pv0), (1, pv1)):
                    nc.tensor.matmul(pv[0:HD + 1, lo:512], vaug[:, t, :],
                                     pt[:, h2, lo:512],
                                     start=(t == 0), stop=(t == T - 1),
                                     skip_group_check=True)
            # epilogue part A (scalar+DVE only): denominators -> reciprocals.
            # Both pv tiles carry data rows 0..63 and the denominator row 64.
            dsa = sbp.tile([1, 512], F32, tag="dsa", name=f"ds{n}{m}")
            nc.scalar.copy(dsa[:], pv0[HD:HD + 1, :])
            dsb = sbp.tile([1, 512], F32, tag="dsb", name=f"db{n}{m}")
            nc.scalar.copy(dsb[:], pv1[HD:HD + 1, :])
            reca = sbp.tile([1, 512], F32, tag="reca", name=f"rr{n}{m}")
            nc.vector.reciprocal_approx_fast(reca[:], dsa[:])
            recb = sbp.tile([1, 512], F32, tag="recb", name=f"rw{n}{m}")
            nc.vector.reciprocal_approx_fast(recb[:], dsb[:])
            rba = sbp.tile([1, 512], BF16, tag="rba", name=f"ba{n}{m}")
            nc.vector.tensor_copy(rba[:], reca[:])
            rbb = sbp.tile([1, 512], BF16, tag="rbb", name=f"bb{n}{m}")
            nc.vector.tensor_copy(rbb[:], recb[:])

            def epi_b():
                rbps = scps.tile([P, 2, 512], F32, tag="sc", name=f"rb{n}{m}")
                nc.tensor.matmul(rbps[0:HD, 0, :], ones1_sb[0:1, :], rba[:],
                                 start=True, stop=True, skip_group_check=True)
                nc.tensor.matmul(rbps[0:HD, 1, :], ones1_sb[0:1, :], rbb[:],
                                 start=True, stop=True, skip_group_check=True)
                rec_s = sbp.tile([HD, 2, 512], BF16, tag="recs",
                                 name=f"rs{n}{m}")
                nc.vector.tensor_copy(rec_s[:], rbps[0:HD, :, :])
                nc.vector.tensor_mul(at_n[0:HD, m, :], pv0[0:HD, :],
                                     rec_s[:, 0, :])
                # odd half: scale at partitions 0..63, then shift to 64..127
                # via the ishift matmul (PSUM col base must be 0/64-aligned)
                tmp = sbp.tile([HD, 512], BF16, tag="oddt", name=f"od{n}{m}")
                nc.vector.tensor_mul(tmp[:], pv1[0:HD, :], rec_s[:, 1, :])
                rp = scps.tile([P, 2, 512], F32, tag="sc", name=f"rp{n}{m}")
                nc.tensor.matmul(rp[:, 0, :], ishift_sb[:], tmp[:],
                                 start=True, stop=True, skip_group_check=True)
                nc.vector.tensor_copy(at_n[HD:P, m, :], rp[HD:P, 0, :])

            return gen, epi_b

        def out_proj(n, at_n):
            for st in range(4):
                po = pjps.tile([P, 1024], F32, tag="pj", name=f"po{n}{st}")
                sl = slice(st * P, (st + 1) * P)
                for m in (0, 1):
                    for nn2 in (0, 1):
                        nc.tensor.matmul(po[:, nn2 * 512:(nn2 + 1) * 512],
                                         at_n[:, m, sl],
                                         wo_sb[:, m, nn2 * 512:(nn2 + 1) * 512],
                                         start=(m == 0), stop=(m == 1),
                                         skip_group_check=True)
                    yield
                ot = otp.tile([P, 1024], BF16, tag="ot", name=f"o{n}{st}")
                nc.vector.tensor_copy(ot[:], po[:])
                nc.sync.dma_start(d_out[(4 * n + st) * P:(4 * n + st + 1) * P, :],
                                  ot[:])
                yield

        def chain_gens(parts):
            for g in parts:
                yield from g

        # ---- fused main loop
        gen = prepare(0)
        while drain(gen) is not None:
            pass
        gen = None
        post = None   # deferred out_proj generator of the previous chunk
        epi = None
        for n in range(NQC):
            parts = []
            if n + 1 < NQC:
                parts.append(prepare(n + 1))
            if post is not None:
                parts.append(post)
            gen = chain_gens(parts) if parts else None
            at_n = atp.tile([P, 2, 512], BF16, tag="at", name=f"at{n}")
            for m in (0, 1):
                gen, epi = attention(n, m, refs[n], at_n, gen, epi)
            while drain(gen) is not None:
                pass
            gen = None
            post = out_proj(n, at_n)
        epi()
        while drain(post) is not None:
            pass

    nc.compile()
    return nc


def make_consts():
    """Host-precomputed constant operands shared by all cores."""
    r128 = np.zeros((P, P), np.float32)
    for mm in range(P):
        hh, dd = mm // HD, mm % HD
        if dd < HD // 2:
            r128[hh * HD + dd + HD // 2, mm] = -1.0
        else:
            r128[hh * HD + dd - HD // 2, mm] = 1.0
    rdup = np.zeros((HD, P), np.float32)
    idup = np.zeros((HD, P), np.float32)
    for mm in range(P):
        dd = mm % HD
        idup[dd, mm] = 1.0
        if dd < HD // 2:
            rdup[dd + HD // 2, mm] = -1.0
        else:
            rdup[dd - HD // 2, mm] = 1.0
    ident = np.zeros((P, HD), np.float32)
    ident[HD:P, :] = np.eye(HD)
    ishift = np.zeros((HD, P), np.float32)
    for kk in range(HD):
        ishift[kk, kk + HD] = 1.0
    tri = np.where(np.arange(P)[:, None] <= np.arange(P)[None, :], 0.0,
                   MASK_NEG).astype(BF)
    return dict(r128=r128.astype(BF), rdup=rdup.astype(BF),
                idup=idup.astype(BF), ident=ident.astype(BF),
                ishift=ishift.astype(BF),
                tri=tri, i128b=np.eye(P).astype(BF),
                ones1c=np.ones((P, HD), BF),
                onesv=np.ones((P, NST), BF))


def prep_in_maps(x, cos, sin, wq, wk, wv, wo):
    """Per-core input maps (shared by kernel() and the test harness)."""
    consts = make_consts()
    cosT = np.ascontiguousarray(np.vstack([cos.T, cos.T])).astype(BF)  # [128,S]
    sinT = np.ascontiguousarray(np.vstack([sin.T, sin.T])).astype(BF)

    in_maps = []
    for core in range(8):
        b, g = core // NKV, core % NKV
        xT = np.ascontiguousarray(x[b].T).astype(BF)                 # [HID, S]
        wqT = np.ascontiguousarray(wq[g * FEAT:(g + 1) * FEAT, :].T).astype(BF)
        wkvT = np.ascontiguousarray(
            np.concatenate([wk[g * HD:(g + 1) * HD, :],
                            wv[g * HD:(g + 1) * HD, :]], axis=0).T).astype(BF)
        woT = np.ascontiguousarray(wo[:, g * FEAT:(g + 1) * FEAT].T).astype(BF)
        in_maps.append(dict(xT=xT, wqT=wqT, wkvT=wkvT, woT=woT,
                            cosT=cosT, sinT=sinT, **consts))
    return in_maps


_PROG = None


def kernel(x, cos, sin, wq, wk, wv, wo):
    global _PROG
    x = np.asarray(x, np.float32)
    cos = np.asarray(cos, np.float32)
    sin = np.asarray(sin, np.float32)
    wq = np.asarray(wq, np.float32)
    wk = np.asarray(wk, np.float32)
    wv = np.asarray(wv, np.float32)
    wo = np.asarray(wo, np.float32)

    in_maps = prep_in_maps(x, cos, sin, wq, wk, wv, wo)

    if _PROG is None:
        _PROG = build_program()
    res = run_bass_kernel_spmd(_PROG, in_maps, core_ids=list(range(8)))

    out = np.zeros((B, S, HID), np.float32)
    for core in range(8):
        out[core // NKV] += np.asarray(res.results[core]["outp"], np.float32)
    return out


if __name__ == "__main__":
    rng = np.random.default_rng(0)
    ins = dict(
        x=rng.standard_normal((B, S, HID), np.float32),
        cos=rng.random((S, HD), np.float32),
        sin=rng.random((S, HD), np.float32),
        wq=rng.standard_normal((HID, HID), np.float32) * HID ** -0.5,
        wk=rng.standard_normal((NKV * HD, HID), np.float32) * HID ** -0.5,
        wv=rng.standard_normal((NKV * HD, HID), np.float32) * HID ** -0.5,
        wo=rng.standard_normal((HID, HID), np.float32) * HID ** -0.5,
    )
    out = kernel(**ins)
    print("kernel ran, out shape", out.shape, "mean", float(np.abs(out).mean()))
